# revision 32
# baseline (speedup 1.0000x reference)
"""Pairwise-BCE event loss (EventLossFunction) on 8 Trainium2 NeuronCores.

loss = -sum_{keep} [ Y*ln(P) + (1-Y)*ln(1-P) ] / (N*(N-1))
  P = S @ S.T  (S: [N,H] softmax rows), Y = (labels_i == labels_j),
  keep kills pairs (i, i mod 1024).

Decomposition (exact):
  total = A + B - Ccorr
    A     = sum_{all i,j} ln(1-P_ij)                    (dense, O(N^2))
    B     = sum_{equal-label pairs} [ln P - ln(1-P)]     (label-group blocks)
    Ccorr = sum_i [(1-m_i) ln(1-q_i) + m_i ln(q_i)]      (pseudo-diagonal)
  q_i = s_i . s_{i mod 1024},  m_i = (labels_i == labels_{i mod 1024})
  loss = -total / (N*(N-1))

Tricks:
  * Symmetry: rows are cut into 16 stripes; stripe r only computes the
    cyclic band of 9 stripe-blocks starting at its own diagonal block.
    Off-band blocks are transposes of computed ones, so the host weighs
    span partials x2 (band interior) or x1 (diag block + the half-band
    block), recovering the full ordered sum at 56% of the element work.
  * K=65 contraction: a constant extra row in lhsT/rhs makes the PE
    accumulate P-1 (dense) or P-0.5*u_i*u_j (group blocks) directly in
    PSUM, so no elementwise pre-bias is ever needed.
  * Dense spans are split between two fused drain pipelines, balanced
    across the scalar and vector engines (PSUM can only be read by
    those two):
      - ACT path: ln(-x) with free-dim accumulation, in-place on PSUM
      - DVE path: PSUM->fp16 scale-copy (per-partition constants c_p
        de-bias the near-1 product rounding; host subtracts wd*ln(c_p)
        exactly), then a 4-level pairwise-product tree (16 (P-1)
        factors per leaf; even count => positive product).  All leaves
        collect in one buffer finished by 1-2 big Ln+accum calls.
  * Group blocks pad with u=0 so padded pairs hit exactly 0.5, whose
    lnP / ln(1-P) contributions cancel in the host-side subtraction.
  * PSUM is managed as 4 x 2-bank rotating tiles; inputs/outputs are
    packed into few tensors (HWDGE dispatch is ~625ns each, serial).
Only per-lane partial sums leave the chip; host reduces in float64.
"""

import numpy as np
import ml_dtypes

import concourse.bass as bass
import concourse.tile as tile
import concourse.mybir as mybir
from concourse import bacc
from concourse.bass_utils import run_bass_kernel_spmd

N_CORES = 8
BATCH = 1024  # module's batch_size; sets the keep-mask pattern
F32 = mybir.dt.float32
BF16 = mybir.dt.bfloat16
FP16 = mybir.dt.float16
LN = mybir.ActivationFunctionType.Ln
X = mybir.AxisListType.X
ADD = mybir.AluOpType.add

_PROG_CACHE = {}
DVE_BIAS = 0.0  # >0 pushes more dense spans onto the DVE tree path
TREE_COUNT = 18  # exact number of tree spans (overrides the greedy)
LEAF_SPLIT = 2 / 3  # emit a partial leaf-ln once this fraction is filled (1.0 = never)
GPS_LEVELS = ()  # tree levels (2,3,4) to run on GPSIMD instead of the DVE
TREE_SQUEEZE = 1.0  # trees occupy only this leading fraction of the stream

N_STRIPES = 16          # row stripes over N; 2 per core
N_BAND = 9              # stripe-blocks computed per stripe (diag + 8)


def _dense_spans(SW, RUNW):
    """(offset, width, weight) spans of one stripe's band run.
    Weight-1 head (diag block), weight-2 interior, weight-1 tail (the
    half-band block, whose transpose is computed by the partner stripe)."""
    spans = [(0, SW, 1)]
    off = SW
    while off < RUNW - SW:
        wd = min(2048, RUNW - SW - off)
        spans.append((off, wd, 2))
        off += wd
    spans.append((RUNW - SW, SW, 1))
    return spans


def _plan(N, H, C):
    dve_bias = DVE_BIAS
    R = N // N_CORES
    SW = N // N_STRIPES
    RUNW = SW * N_BAND
    NRT = -(-SW // 128)
    assert SW % NRT == 0
    RT = SW // NRT
    SP = _dense_spans(SW, RUNW)
    NROWT = 2 * NRT

    def act_cost(n):
        return n * 0.8333 + 185.0

    def dve_cost(n):  # PSUM->fp16 scale-copy + 4-level product tree
        c = n * 1.0417 + 125.0
        m = n // 2
        while m >= n // 16:
            c += m * 0.5208 + 60.0
            m //= 2
        return c

    G = 64 // N_CORES
    HB = C // 128
    act_load = HB * 2 * act_cost(G * 256) + 2 * act_cost(NRT) + 400.0
    dve_load = 2 * (NRT * H) * 1.0417 + 800.0
    use_tree = {}
    for rt in range(NROWT):
        for si, (off, wd, wt_) in enumerate(SP):
            treeable = wd % 16 == 0 and wt_ == 2
            a_direct = act_load + act_cost(wd)
            a_tree = max(act_load + (wd // 16) * 0.8333, dve_load + dve_cost(wd))
            if treeable and a_tree < a_direct + (dve_bias or 0.0):
                use_tree[(rt, si)] = True
                act_load += (wd // 16) * 0.8333
                dve_load += dve_cost(wd)
            else:
                use_tree[(rt, si)] = False
                act_load += act_cost(wd)
    if TREE_COUNT is not None:
        elig = [k for k in sorted(use_tree)
                if SP[k[1]][1] % 16 == 0 and SP[k[1]][2] == 2]
        for i, k in enumerate(elig):
            use_tree[k] = (i * TREE_COUNT) // len(elig) !=                           ((i + 1) * TREE_COUNT) // len(elig)
    return R, SW, RUNW, NRT, RT, SP, NROWT, use_tree


def _cvec(RT):
    p = np.arange(RT)
    return (1.05 + 0.25 * ((p * 37) % 64) / 64.0).astype(np.float32)


def _build_program(N, H, C):
    """One SPMD program for all 8 cores.  C = padded label-group width."""
    R, SW, RUNW, NRT, RT, SP, NROWT, use_tree = _plan(N, H, C)
    G = 64 // N_CORES             # label groups per core
    HB = C // 128                 # 128-row halves per group block
    K = H + 1                     # contraction with the constant row
    NSP = len(SP)

    nc = bacc.Bacc("TRN2", target_bir_lowering=False, debug=False)

    run_d = nc.dram_tensor("strun", [K, 2 * RUNW], BF16, kind="ExternalInput").ap()
    stl_d = nc.dram_tensor("stl2", [K, R], BF16, kind="ExternalInput").ap()
    sg_d = nc.dram_tensor("stg", [K, 2 * G * C], BF16, kind="ExternalInput").ap()
    srm_d = nc.dram_tensor("srowsmod", [R, 2 * H], BF16, kind="ExternalInput").ap()
    wcv_d = nc.dram_tensor("wcv", [RT, 4 * NRT + 1], F32, kind="ExternalInput").ap()

    PA_W = 2 * NROWT * NSP + 2
    NBQ = HB * (G * 256 // 1024)
    out_d = nc.dram_tensor("out", [128, PA_W + 2 * NBQ + 2], F32,
                           kind="ExternalOutput").ap()

    with tile.TileContext(nc) as tc:
        with (
            tc.tile_pool(name="const", bufs=1) as cp,
            tc.tile_pool(name="scr", bufs=3) as scr,
            tc.tile_pool(name="tree", bufs=2) as tp,
            tc.tile_pool(name="psum", bufs=4, space="PSUM") as pp,
        ):
            # ---------------- input loads ----------------
            stl_t = cp.tile([K, R], BF16)
            nc.sync.dma_start(out=stl_t[:, :], in_=stl_d[:, :])
            # band runs in two pieces per stripe: early spans, then the rest
            splitp = SP[2][0] if NSP > 2 else RUNW
            ch = {}
            run_t = {}
            halves = [(0, splitp), (splitp, RUNW)]
            def load_run(sidx, half):
                o0, o1 = halves[half]
                c = cp.tile([K, o1 - o0], BF16, name=f"run{sidx}_{half}")
                nc.sync.dma_start(
                    out=c[:, :],
                    in_=run_d[:, sidx * RUNW + o0:sidx * RUNW + o1])
                run_t[(sidx, half)] = c
            load_run(0, 0)
            wcv_t = cp.tile([RT, 4 * NRT + 1], F32)
            nc.sync.dma_start(out=wcv_t[:, :], in_=wcv_d[:, :])
            w1_t = wcv_t[:, 0:2 * NRT]
            w2_t = wcv_t[:, 2 * NRT:4 * NRT]
            cv_t = wcv_t[:, 4 * NRT:4 * NRT + 1]
            load_run(0, 1)
            sg_t = cp.tile([K, 2 * G * C], BF16)
            nc.sync.dma_start(out=sg_t[:, :], in_=sg_d[:, :])
            load_run(1, 0)
            load_run(1, 1)
            for si, (off, wd, _) in enumerate(SP):
                for sidx in range(2):
                    half = 0 if off < splitp else 1
                    base = 0 if half == 0 else splitp
                    ch[(sidx, si)] = run_t[(sidx, half)][:, off - base:off - base + wd]
            sgl_t = sg_t[:, 0:G * C]
            sgr_t = sg_t[:, G * C:2 * G * C]
            srm_t = cp.tile([RT, 2 * NRT, 2 * H], BF16)
            nc.sync.dma_start(out=srm_t[:, :, :],
                              in_=srm_d.rearrange("(t p) k -> p t k", p=RT))
            sr_t = srm_t[:, :, 0:H]
            sm_t = srm_t[:, :, H:2 * H]

            # ---------------- term B: label-group blocks ----------------
            # PSUM x = P - 0.5*u_i*u_j via K=65.  Real pairs: x = P - 0.5;
            # pads: x = 0.  ln(x+0.5) = ln(Pt), ln(-x+0.5) = ln(1-Pt);
            # pads give ln(.5) in both -> cancel in pbp - pbm.
            out_t = cp.tile([128, PA_W + 2 * NBQ + 2], F32)
            nc.vector.memset(out_t[:, :], 0.0)
            pbp_t = out_t[:, PA_W:PA_W + NBQ]
            pbm_t = out_t[:, PA_W + NBQ:PA_W + 2 * NBQ]
            b05 = cp.tile([128, 1], F32)
            nc.vector.memset(b05[:, :], 0.5)

            GQ = G * 256 // 1024          # B sub-tiles per half
            GL = G // GQ                   # labels per sub-tile

            def emit_b(hq):
                h, q = hq
                ps = pp.tile([128, 1024], F32, tag="ps", name=f"bps_{h}_{q}")
                for gi in range(GL):
                    g = q * GL + gi
                    lo = g * C + h * 128
                    nc.tensor.matmul(out=ps[:, gi * 256:(gi + 1) * 256],
                                     lhsT=sgl_t[:, lo:lo + 128],
                                     rhs=sgr_t[:, g * C:g * C + 256],
                                     start=True, stop=True)
                col = h * GQ + q
                ob2 = scr.tile([128, 1024], BF16, tag="bscr", name=f"bq_{h}_{q}")
                nc.scalar.activation(ob2[:, :], ps[:, :], LN,
                                     bias=b05[:, :], scale=-1.0,
                                     accum_out=pbm_t[:, col:col + 1])
                nc.scalar.activation(ps[:, :], ps[:, :], LN,
                                     bias=b05[:, :], scale=1.0,
                                     accum_out=pbp_t[:, col:col + 1])

            # ---------------- dense term A (banded) ----------------
            # PSUM x = P - 1 via K=65.
            pa_t = out_t[:, 0:PA_W]
            n_leaf = sum(SP[si][1] // 16 for (rt, si), v in use_tree.items() if v)
            for (rt, si), v in use_tree.items():
                if v:
                    assert SP[si][2] == 2  # combined-ln column is weighted x2
            lf = cp.tile([128, max(n_leaf, 16)], FP16)
            oL1 = scr.tile([128, max(n_leaf, 16)], BF16, tag="oL")
            leaf_off = [0]
            leaf_done = [0]
            # evenly interleave tree spans, direct spans, and B tiles so
            # the scalar and vector engines never starve each other
            trees = [k for k in sorted(use_tree) if use_tree[k]]
            directs = [k for k in sorted(use_tree) if not use_tree[k]]
            stream = []
            for i, k in enumerate(trees):
                stream.append((TREE_SQUEEZE * (i + 0.5) / max(len(trees), 1),
                               "d", k))
            for i, k in enumerate(directs):
                stream.append(((i + 0.5) / max(len(directs), 1), "d", k))
            bitems = [(h, q) for h in range(HB) for q in range(GQ)]
            for i, hq in enumerate(bitems):
                stream.append(((i + 0.5) / len(bitems), "b", hq))
            stream.sort(key=lambda x: (x[0], x[1]))
            for _, kind, key in stream:
                if kind == "b":
                    emit_b(key)
                    continue
                rt, si = key
                off, wd, _ = SP[si]
                sidx = rt // NRT
                lh = stl_t[:, rt * RT:(rt + 1) * RT]
                if True:
                    idx = rt * NSP + si
                    tree = use_tree[(rt, si)]
                    t0 = None
                    if tree:
                        t0 = tp.tile([128, 2048], FP16, tag="t0", name=f"t0_{rt}_{si}")
                    pieces = [(p0, min(1024, wd - p0)) for p0 in range(0, wd, 1024)]
                    for pi, (p0, pw) in enumerate(pieces):
                        ps = pp.tile([128, 1024], F32, tag="ps",
                                     name=f"dps_{rt}_{si}_{pi}")
                        for j0 in range(0, pw, 512):
                            jw = min(512, pw - j0)
                            nc.tensor.matmul(
                                out=ps[0:RT, j0:j0 + jw], lhsT=lh,
                                rhs=ch[(sidx, si)][:, p0 + j0:p0 + j0 + jw],
                                start=True, stop=True)
                        if tree:
                            nc.vector.tensor_scalar_mul(
                                t0[0:RT, p0:p0 + pw], ps[0:RT, 0:pw],
                                cv_t[0:RT, :])
                        else:
                            nc.scalar.activation(
                                ps[0:RT, 0:pw], ps[0:RT, 0:pw], LN,
                                bias=0.0, scale=-1.0,
                                accum_out=pa_t[0:RT, 2 * idx + pi:2 * idx + pi + 1])
                    if len(pieces) < 2 and not tree:
                        nc.vector.memset(pa_t[:, 2 * idx + 1:2 * idx + 2], 0.0)
                    if tree:
                        if (leaf_done[0] == 0
                                and leaf_off[0] >= int(n_leaf * LEAF_SPLIT)):
                            mk = leaf_off[0]
                            leaf_done[0] = mk
                            nc.scalar.activation(
                                oL1[0:RT, 0:mk], lf[0:RT, 0:mk], LN,
                                bias=0.0, scale=1.0,
                                accum_out=pa_t[0:RT,
                                               2 * NROWT * NSP:2 * NROWT * NSP + 1])
                        w2_, w4, w8, w16 = wd // 2, wd // 4, wd // 8, wd // 16
                        t1 = tp.tile([128, 1024], FP16, tag="t1", name=f"t1_{rt}_{si}")
                        nc.vector.tensor_mul(t1[0:RT, 0:w2_], t0[0:RT, 0:w2_],
                                             t0[0:RT, w2_:wd])
                        e2 = nc.gpsimd if 2 in GPS_LEVELS else nc.vector
                        e3 = nc.gpsimd if 3 in GPS_LEVELS else nc.vector
                        e4 = nc.gpsimd if 4 in GPS_LEVELS else nc.vector
                        t2 = tp.tile([128, 512], FP16, tag="t2", name=f"t2_{rt}_{si}")
                        e2.tensor_mul(t2[0:RT, 0:w4], t1[0:RT, 0:w4],
                                      t1[0:RT, w4:w2_])
                        t3 = tp.tile([128, 256], FP16, tag="t3", name=f"t3_{rt}_{si}")
                        e3.tensor_mul(t3[0:RT, 0:w8], t2[0:RT, 0:w8],
                                      t2[0:RT, w8:w4])
                        lo4 = leaf_off[0]
                        leaf_off[0] = lo4 + w16
                        e4.tensor_mul(lf[0:RT, lo4:lo4 + w16],
                                      t3[0:RT, 0:w16], t3[0:RT, w16:w8])
            # ---------------- pseudo-diagonal corrections ----------------
            prod = cp.tile([RT, 2 * NRT, H], F32)
            nc.vector.tensor_mul(prod[:, :, :], sr_t[:, :, :], sm_t[:, :, :])
            q_t = cp.tile([RT, 2 * NRT], F32)
            nc.vector.tensor_reduce(q_t[:, :], prod[:, :, :], axis=X, op=ADD)
            l1q = cp.tile([RT, 2 * NRT], F32)
            nc.scalar.activation(l1q[:, :], q_t[:, :], LN, bias=1.0, scale=-1.0)
            lnq = cp.tile([RT, 2 * NRT], F32)
            nc.scalar.activation(lnq[:, :], q_t[:, :], LN, bias=0.0, scale=1.0)
            jk1 = cp.tile([RT, 2 * NRT], F32)
            c1_t = out_t[0:RT, PA_W + 2 * NBQ:PA_W + 2 * NBQ + 1]
            nc.vector.tensor_mul(jk1[:, :], l1q[:, :], w1_t[:, :])
            nc.vector.tensor_reduce(c1_t, jk1[:, :], axis=X, op=ADD)
            jk2 = cp.tile([RT, 2 * NRT], F32)
            c2_t = out_t[0:RT, PA_W + 2 * NBQ + 1:PA_W + 2 * NBQ + 2]
            nc.vector.tensor_mul(jk2[:, :], lnq[:, :], w2_t[:, :])
            nc.vector.tensor_reduce(c2_t, jk2[:, :], axis=X, op=ADD)

            nb = 2 * NROWT * NSP
            if n_leaf:
                mk = leaf_done[0]
                if mk == 0:
                    nc.vector.memset(pa_t[:, nb:nb + 1], 0.0)
                oL = scr.tile([128, max(n_leaf, 16)], BF16, tag="oL2")
                nc.scalar.activation(
                    oL[0:RT, 0:n_leaf - mk], lf[0:RT, mk:n_leaf], LN,
                    bias=0.0, scale=1.0,
                    accum_out=pa_t[0:RT, nb + 1:nb + 2])
            else:
                nc.vector.memset(pa_t[:, nb:nb + 2], 0.0)
            nc.sync.dma_start(out=out_d[:, :], in_=out_t[:, :])

    nc.compile()
    return nc


def kernel(softmax_output, labels):
    S = np.asarray(softmax_output, dtype=np.float32)
    lab = np.asarray(labels).astype(np.int64)
    N, H = S.shape
    assert N % (N_CORES * 2 * 128) == 0 or N % N_STRIPES == 0
    R = N // N_CORES
    SW = N // N_STRIPES
    RUNW = SW * N_BAND
    n_lab = 64
    G = n_lab // N_CORES

    counts = np.bincount(lab, minlength=n_lab)
    C = max(256, int(-(-counts.max() // 128)) * 128)
    # the group-block pipeline packs 4 blocks of 256 cols per PSUM tile
    assert C == 256, f"label group > 256 rows unsupported (max {counts.max()})"

    key = (N, H, C)
    if key not in _PROG_CACHE:
        _PROG_CACHE[key] = _build_program(N, H, C)
    nc = _PROG_CACHE[key]

    _, _, _, NRTp, RTp, SPp, NROWTp, use_tree = _plan(N, H, C)
    cv = _cvec(RTp)
    ST = np.ascontiguousarray(S.T).astype(ml_dtypes.bfloat16)       # [H, N]
    S_bf = S.astype(ml_dtypes.bfloat16)                              # [N, H]

    # dense operands with the constant K=65 row
    ST2 = np.concatenate(
        [ST, np.full((1, N), -1.0, dtype=ml_dtypes.bfloat16)], axis=0)
    # cyclic band runs, one per stripe
    band_idx = (np.arange(RUNW)[None, :] +
                SW * np.arange(N_STRIPES)[:, None]) % N    # [16, RUNW]

    # label-group gather, padded to C columns per label
    order = np.argsort(lab, kind="stable")
    STg = np.zeros((H, n_lab * C), dtype=ml_dtypes.bfloat16)
    U = np.zeros((n_lab * C,), dtype=np.float32)
    pos = 0
    for l in range(n_lab):
        c = int(counts[l])
        cols = order[pos:pos + c]
        pos += c
        STg[:, l * C:l * C + c] = ST[:, cols]
        U[l * C:l * C + c] = 1.0
    STgL = np.concatenate(
        [STg, (-0.5 * U).astype(ml_dtypes.bfloat16)[None, :]], axis=0)
    STgR = np.concatenate(
        [STg, U.astype(ml_dtypes.bfloat16)[None, :]], axis=0)

    mod_idx = np.arange(N) % BATCH
    m = (lab == lab[mod_idx]).astype(np.float32)
    w1 = 1.0 - m
    w2 = m
    Smod = S_bf[mod_idx]

    ones_row = np.ones((1, R), dtype=ml_dtypes.bfloat16)
    in_maps = []
    for k in range(N_CORES):
        rs = slice(k * R, (k + 1) * R)
        gs = slice(k * G * C, (k + 1) * G * C)
        runs = np.concatenate(
            [ST2[:, band_idx[2 * k]], ST2[:, band_idx[2 * k + 1]]], axis=1)
        NRT2 = NROWTp
        w1p = w1[rs].reshape(NRT2, RTp).T
        w2p = w2[rs].reshape(NRT2, RTp).T
        wcv = np.concatenate([w1p, w2p, cv[:, None]], axis=1).astype(np.float32)
        in_maps.append({
            "strun": np.ascontiguousarray(runs),
            "stl2": np.ascontiguousarray(
                np.concatenate([ST[:, rs], ones_row], axis=0)),
            "stg": np.ascontiguousarray(
                np.concatenate([STgL[:, gs], STgR[:, gs]], axis=1)),
            "srowsmod": np.ascontiguousarray(
                np.concatenate([S_bf[rs], Smod[rs]], axis=1)),
            "wcv": np.ascontiguousarray(wcv),
        })

    res = run_bass_kernel_spmd(nc, in_maps, list(range(N_CORES)))

    # per-span symmetry weights, matching the program's pa layout
    SP = SPp
    wts = np.repeat(np.array([w for _, _, w in SP], dtype=np.float64), 2)
    span_w = np.concatenate([np.tile(wts, 2 * NRTp), [2.0, 2.0]])

    # tree spans carry an extra wd * sum_p ln(c_p) from the de-bias scaling
    lncsum = np.log(cv.astype(np.float64)).sum()
    tree_corr = 0.0
    for rt in range(NROWTp):
        for si, (off, wd, w) in enumerate(SP):
            if use_tree[(rt, si)]:
                tree_corr += w * wd * lncsum

    PA_W = len(span_w)
    total = -tree_corr * N_CORES
    for r in res.results:
        o = r["out"].astype(np.float64)
        nbq = (o.shape[1] - PA_W - 2) // 2
        total += (o[0:RTp, 0:PA_W] * span_w[None, :]).sum()
        total += o[:, PA_W:PA_W + nbq].sum()
        total -= o[:, PA_W + nbq:PA_W + 2 * nbq].sum()
        total -= o[0:RTp, PA_W + 2 * nbq:].sum()

    loss = -total / float(N * (N - 1))
    return np.asarray(loss, dtype=np.float32)
